# revision 31
# baseline (speedup 1.0000x reference)
"""Trainium2 Bass kernel for histogram_binning (windowed-cosine binning).

Reference computation (per element):
    d = x[k,i] - phis[i,j]
    out[k, i*L+j] = 0.5*cos(d)+0.5  if  -interval[i] < d <= interval[i]  else 0

Strategy (8 cores, data-parallel over batch; each core owns 128 batch rows):
  - Polynomial value path (no ACT sin): with s = 48**-0.25 and t' = (s*d)^2,
        0.5*cos(d)+0.5  ~=  (t' - sqrt(3)/2)^2 + 1/4         (|err| <= 7e-4)
    and the window test becomes  t' <= (s*iv)^2  (exact up to ~5e-7 relative
    slack at the window edge; the harness tolerance is 2e-2 L2).
    Host pre-scales the tiny inputs: xts = s*x^T, phs = s*phis, ivq = (s*iv)^2.
  - On-chip layout: partition dim = feature i (two 128-halves), free dim =
    (k_block, j).  phs half [128,256], ivq half [128,1], xts half [128,128]
    stay resident.
  - Two fused compute routes per chunk (mixed to balance DVE vs ACT):
    route A (DVE-only, "xc"): ONE custom DVE instruction per K-row chunk
      with a hand-written 3-state subdim FSM: in0 streams the phi tile K
      times via a stride-0 page dim; block 0's swap flop holds the current
      row's x (reloaded from in1 at each SUB_DIM_DONE); computes
      out = (t'<=ivq) * (t'-sqrt3/2)^2 fused.  ~304 ns/row vs ~553 for
      per-row ops (amortizes the ~280 ns DVE instruction overhead).
    route B (ACT-assisted): ACT Square computes t' = (xs - phi_s)^2 per row
      (bias = xs column, scale = -1); one 4-stage custom DVE op with a
      hand-written 2x_2P perf-mode program (2 elem/cycle, write0=chainA /
      write1=chainB like the stock fp32 tensor_scalar 2x program) windows +
      polys the whole chunk: out = (u<=ivq-sqrt3/2) * u^2, u = t'-sqrt3/2.
  - Device output is v = val - 1/4 (in-window) or 0; the host restores the
    +1/4 under the (out != 0) mask.  Output dtype is bf16 (halves the
    HBM-write floor; +2e-3 rel err), upcast to f32 on the host.
  - Measured: ~56 us vs 115.3 us baseline (rel err ~1.3e-3, gate 2e-2).
"""

import math
import os

import numpy as np

import concourse.bacc as bacc
import concourse.mybir as mybir
from concourse import dve_ops
from concourse.bass_utils import run_bass_kernel_spmd
from concourse.dve_spec import (
    C0,
    C1,
    C2,
    C3,
    Spec,
    Src0,
    _has_src1,
    _spill_c3_to_src1,
    lower,
    sq,
)
from concourse.dve_uop import (
    DISABLE,
    ENABLE,
    AluInp,
    AluOp,
    DelayInp,
    DveOpSpec,
    InpSel,
    OutPath,
    OutSel,
    Trigger,
    UopConfig,
    UopDpConfig,
)
from concourse.tile import TileContext

B, M, L = 1024, 256, 256
N_CORES = 8
B_SHARD = B // N_CORES  # 128
HALF = 128  # features per partition-half
F32 = mybir.dt.float32
BF16 = mybir.dt.bfloat16

SCALE = float(48.0 ** -0.25)  # s: makes the poly monic in t'
C_SQ32 = float(math.sqrt(3.0) / 2.0)  # imm2
C_QTR = 0.25  # s1

_OPS_CACHE = {}


def _register_op(name, spec, subdim=False):
    """Register a custom DVE op under `name`, computing its uops sha."""
    if name in _OPS_CACHE:
        return _OPS_CACHE[name]
    for existing in dve_ops.OPS:
        if existing.name == name:
            _OPS_CACHE[name] = existing
            return existing
    if name not in dve_ops._SUB_OPCODE_FOR_NAME:
        row = max(dve_ops._SUB_OPCODE_FOR_NAME.values()) + 1
        assert row < 0x20, "no free custom-DVE opcode rows"
        dve_ops._SUB_OPCODE_FOR_NAME[name] = row
    shas = {}
    for ver in ("v3", "v4"):
        uops = lower(spec, ver=ver)
        shas[ver] = DveOpSpec(
            name=name,
            opcode=dve_ops.get_dve_sub_opcode(name),
            uops=uops,
            rd1_en=_has_src1(spec),
        ).sha(ver)
    op = dve_ops.DveOp(name, spec, subdim=subdim, uops_sha=shas)
    dve_ops.OPS.append(op)
    dve_ops.CUSTOM_DVE_SPECS[name] = spec
    _OPS_CACHE[name] = op
    return op


def _get_winpoly_full_op():
    """Fully fused per-row op.  Src0 = s*phi, C3 (in1, [P,1]) = s*x,
    C0 = (s*iv)^2 [P,1], C2 (imm2) = sqrt(3)/2.
        d  = C3 - Src0            t' = d^2
        out = (t' <= C0) * (t' - C2)^2        (the +1/4 is host-side)
    6 ALU stages."""
    d = C3 - Src0
    t = sq(d)
    cond = t <= C0
    body = cond * sq(t - C2)
    body = _spill_c3_to_src1(body)

    def _ref(in0, in1, s0, s1, imm2):
        f = np.float32
        d = (in1 - in0).astype(f)
        t = (d * d).astype(f)
        cond = t <= s0
        u = (t - f(imm2)).astype(f)
        v = (u * u).astype(f)
        return (cond.astype(f) * v).astype(f)

    return _register_op("WINPOLY_VMQ_ANT", Spec(body=body, reference=_ref))


def _get_winpoly_vmq2_op():
    """Latch-free fused per-row op (single uop state, rd1 free).
    Src0 = s*phi, C1 (s1, [P,1]) = s*x, C0 = (s*iv)^2 [P,1],
    C2 (imm2) = sqrt(3)/2.
        d = C1 - Src0 ; t' = d^2 ; out = (t' <= C0) * (t' - C2)^2
    6 ALU stages; the +1/4 is restored host-side."""
    d = C1 - Src0
    t = sq(d)
    cond = t <= C0
    body = cond * sq(t - C2)

    def _ref(in0, in1, s0, s1, imm2):
        f = np.float32
        d = (s1 - in0).astype(f)
        t = (d * d).astype(f)
        cond = t <= s0
        u = (t - f(imm2)).astype(f)
        v = (u * u).astype(f)
        return (cond.astype(f) * v).astype(f)

    return _register_op("WINPOLY_VMQ2_ANT", Spec(body=body, reference=_ref))


def _build_xc_uops():
    """3-state FSM for the x-cycling fused op (subdim pages over in0).

    in0 = phi tile streamed K times via a stride-0 page dim [P, K, N];
    in1 = [P, K] per-page x values.  Block 0's swap flop holds the current
    page's x; SUB_DIM_DONE hops to a step state that reloads it from Src1
    while processing that page's first element.
        d = x - phi ; t = d^2 ; out = (t <= C0) * (t - C2)^2
    """
    P, SW = AluInp.PREV_ALU_OUT, AluInp.CURR_SWAP_OUT
    D = [AluInp.PREV_DELAY_0, AluInp.PREV_DELAY_1, AluInp.PREV_DELAY_2,
         AluInp.PREV_DELAY_3, AluInp.PREV_DELAY_4, AluInp.PREV_DELAY_5]

    # state 0: init -- swap-load x[0] from Src1, no compute, no output
    init = UopConfig()
    init.enable_input(InpSel.SRC_1, 1)  # -> lane 0
    init.require_inp1 = ENABLE
    init.trigger = (Trigger.COUNT, Trigger.NONE, Trigger.NONE)
    init.repeat_count = 1
    init.next_uop = (1, 0, 0)
    b = init.datapath_config
    b[0].enable_alu(AluOp.BYPASS, D[0], D[0])
    b[0].swap_enable = ENABLE

    # state 1: steady -- d = swap - Src0, windowed poly, write out
    st = UopConfig()
    st.enable_input(InpSel.SRC_0, 0)
    st.enable_input(InpSel.CONST_0, 1)  # ivq -> lane 0
    st.enable_input(InpSel.CONST_2, 2)  # sqrt3/2 -> lane 1
    st.require_inp0 = ENABLE
    st.trigger = (Trigger.SRC_TENSOR_DONE, Trigger.SUB_DIM_DONE, Trigger.NONE)
    st.next_uop = (0, 2, 0)
    st.enable_output(OutSel.ALU_OUT, OutPath.WR0_LO)
    b = st.datapath_config
    b[0].enable_alu(AluOp.SUBTRACT, SW, P).pass_through_delay(0, 1)
    b[1].enable_alu(AluOp.MULTIPLY, P, P).pass_through_delay(0, 1)
    b[2].enable_alu(AluOp.IS_GE, D[0], P).pass_through_delay(1)
    b[2].enable_delay_from_src(DelayInp.PREV_ALU_OUT, 2)  # t
    b[3].enable_alu(AluOp.SUBTRACT, D[2], D[1])
    b[3].enable_delay_from_src(DelayInp.PREV_ALU_OUT, 3)  # cond
    b[4].enable_alu(AluOp.MULTIPLY, P, P).pass_through_delay(3)
    b[5].enable_alu(AluOp.MULTIPLY, P, D[3])
    b[6].pass_through_alu()
    b[7].pass_through_alu()

    # state 2: step -- swap-load next x AND process the page's first element
    sp = UopConfig()
    sp.enable_input(InpSel.SRC_1, 1)   # x  -> lane 0
    sp.enable_input(InpSel.SRC_0, 2)   # phi -> lane 1
    sp.enable_input(InpSel.CONST_0, 3)  # ivq -> lane 2
    sp.enable_input(InpSel.CONST_2, 4)  # sqrt3/2 -> lane 3
    sp.require_inp0 = ENABLE
    sp.require_inp1 = ENABLE
    sp.trigger = (Trigger.SRC_TENSOR_DONE, Trigger.SUB_DIM_DONE, Trigger.COUNT)
    sp.next_uop = (0, 2, 1)
    sp.repeat_count = 1
    sp.enable_output(OutSel.ALU_OUT, OutPath.WR0_LO)
    b = sp.datapath_config
    b[0].enable_alu(AluOp.BYPASS, D[0], D[0]).pass_through_delay(1, 2, 3)
    b[0].swap_enable = ENABLE
    b[1].enable_alu(AluOp.SUBTRACT, P, D[1]).pass_through_delay(2, 3)
    b[2].enable_alu(AluOp.MULTIPLY, P, P).pass_through_delay(2, 3)
    b[3].enable_alu(AluOp.IS_GE, D[2], P).pass_through_delay(3)
    b[3].enable_delay_from_src(DelayInp.PREV_ALU_OUT, 4)  # t
    b[4].enable_alu(AluOp.SUBTRACT, D[4], D[3])
    b[4].enable_delay_from_src(DelayInp.PREV_ALU_OUT, 5)  # cond
    b[5].enable_alu(AluOp.MULTIPLY, P, P).pass_through_delay(5)
    b[6].enable_alu(AluOp.MULTIPLY, P, D[5])
    b[7].pass_through_alu()
    return [init, st, sp]


def _get_winpoly_xc_op():
    """x-cycling fused op: one instruction per K-row chunk.
    in0 = s*phi [P, K(stride-0), N], in1 = s*x [P, K] (one per page),
    C0 = (s*iv)^2 [P,1], C2 (imm2) = sqrt(3)/2.
        d = x_page - phi ; t' = d^2 ; out = (t' <= C0) * (t' - C2)^2"""
    name = "WINPOLY_XC_ANT"
    # Lowerable stand-in body with the same leaf set (the runtime table is
    # the hand-written FSM injected below).
    d = C3 - Src0
    t = sq(d)
    body = _spill_c3_to_src1((t <= C0) * sq(t - C2))

    def _ref(in0, in1, s0, s1, imm2):
        f = np.float32
        a0 = np.asarray(in0, dtype=f)
        x = np.asarray(in1, dtype=f)
        if a0.ndim == 3:
            Pd, S, N = a0.shape
            xx = x.reshape(Pd, S, 1)
            c0 = np.asarray(s0, dtype=f).reshape(Pd, 1, 1) if np.ndim(s0) else f(s0)
        else:
            xx = x
            c0 = s0
        dd = (xx - a0).astype(f)
        tt = (dd * dd).astype(f)
        cond = tt <= c0
        u = (tt - f(imm2)).astype(f)
        v = (u * u).astype(f)
        return (cond.astype(f) * v).astype(f)

    op = _register_op(name, Spec(body=body, reference=_ref), subdim=True)
    spec_v3 = DveOpSpec(
        name=name,
        opcode=dve_ops.get_dve_sub_opcode(name),
        uops=_build_xc_uops(),
        rd1_en=True,
    )
    dve_ops._COMPILE_CACHE[(name, "v3")] = spec_v3
    return op


def _get_winpoly_t_op():
    """Window + poly from precomputed t' (Src0).  C0 = (s*iv)^2 [P,1],
    C2 (imm2) = sqrt(3)/2, C1 (s1) = 1/4.  5 ALU stages."""
    cond = Src0 <= C0
    body = cond * (sq(Src0 - C2) + C1)

    def _ref(in0, in1, s0, s1, imm2):
        f = np.float32
        cond = in0 <= s0
        u = (in0 - f(imm2)).astype(f)
        w = ((u * u).astype(f) + f(s1)).astype(f)
        return (cond.astype(f) * w).astype(f)

    return _register_op("WINPOLY_T_ANT", Spec(body=body, reference=_ref))


def _build_q2x_uop():
    """Hand-written 2x_2P (two results/cycle) program for WINPOLY_Q_ANT.

    Per cycle the engine reads in0[2i] via rd0 (-> SRC_0) and in0[2i+1] via
    rd1 (-> SRC_1), same convention as the stock fp32 tensor_scalar 2x_2P
    program (write0_lo = chain-A result, write1_lo = chain-B result).

    Chain A (blocks 0-3) and chain B (blocks 4-7) each compute
        u = elem - C2 ; cond = (u <= C0) ; v = u*u ; out = cond * v
    Lane plan:
      lane0: elemB (SRC_1 via inp1), consumed at b4
      lane1: C0 (inp2), read at b1 and b5
      lane2: C2 (inp3), read at b0 and b4
      lane3: uA (cap b1, read b2); condB (cap b6, read b7)
      lane4: condA (cap b2, read b3); outA (cap b4, ride to write mux)
      lane5: uB (cap b5, read b6)
    """
    u = UopConfig()
    u.enable_input(InpSel.SRC_0, 0)
    u.enable_input(InpSel.SRC_1, 1)   # -> lane 0
    u.enable_input(InpSel.CONST_0, 2)  # -> lane 1
    u.enable_input(InpSel.CONST_2, 3)  # -> lane 2
    u.require_inp0 = ENABLE
    u.require_inp1 = ENABLE
    u.trigger = (Trigger.SRC_TENSOR_DONE, Trigger.NONE, Trigger.NONE)
    u.next_uop = (0, 0, 0)
    b = u.datapath_config
    P = AluInp.PREV_ALU_OUT
    D = [AluInp.PREV_DELAY_0, AluInp.PREV_DELAY_1, AluInp.PREV_DELAY_2,
         AluInp.PREV_DELAY_3, AluInp.PREV_DELAY_4, AluInp.PREV_DELAY_5]
    # b0: uA = SRC_0 - C2
    b[0].enable_alu(AluOp.SUBTRACT, P, D[2]).pass_through_delay(0, 1, 2)
    # b1: condA = (C0 >= uA); capture uA -> lane3
    b[1].enable_alu(AluOp.IS_GE, D[1], P).pass_through_delay(0, 1, 2)
    b[1].enable_delay_from_src(DelayInp.PREV_ALU_OUT, 3)
    # b2: vA = uA * uA; capture condA -> lane4
    b[2].enable_alu(AluOp.MULTIPLY, D[3], D[3]).pass_through_delay(0, 1, 2)
    b[2].enable_delay_from_src(DelayInp.PREV_ALU_OUT, 4)
    # b3: outA = vA * condA
    b[3].enable_alu(AluOp.MULTIPLY, P, D[4]).pass_through_delay(0, 1, 2)
    # b4: uB = elemB - C2; capture outA -> lane4
    b[4].enable_alu(AluOp.SUBTRACT, D[0], D[2]).pass_through_delay(1)
    b[4].enable_delay_from_src(DelayInp.PREV_ALU_OUT, 4)
    # b5: condB = (C0 >= uB); capture uB -> lane5
    b[5].enable_alu(AluOp.IS_GE, D[1], P).pass_through_delay(4)
    b[5].enable_delay_from_src(DelayInp.PREV_ALU_OUT, 5)
    # b6: vB = uB * uB; capture condB -> lane3
    b[6].enable_alu(AluOp.MULTIPLY, D[5], D[5]).pass_through_delay(4)
    b[6].enable_delay_from_src(DelayInp.PREV_ALU_OUT, 3)
    # b7: outB = vB * condB; outA rides lane4 to the write mux
    b[7].enable_alu(AluOp.MULTIPLY, P, D[3]).pass_through_delay(4)
    u.enable_output(OutSel.DELAY_4, OutPath.WR0_LO)  # result A
    u.enable_output(OutSel.ALU_OUT, OutPath.WR1_LO)  # result B
    return u


def _get_winpoly_q_op():
    """Window + poly-minus-quarter from t' (Src0), 4 ALU stages, with a
    hand-written 2x_2P perf-mode program (2 elem/cycle for fp32 SBUF src).
        u = Src0 - C2 ; out = (u <= C0) * u^2
    C0 = (s*iv)^2 - sqrt(3)/2 [P,1] (shifted window bound; compare on u is
    equivalent to t' <= (s*iv)^2 by monotonicity), C2 (imm2) = sqrt(3)/2.
    The missing +1/4 is restored on the host (out != 0 marks the window)."""
    name = "WINPOLY_Q_ANT"
    u = Src0 - C2
    cond = u <= C0
    body = cond * sq(u)

    def _ref(in0, in1, s0, s1, imm2):
        f = np.float32
        uu = (in0 - f(imm2)).astype(f)
        cond = uu <= s0
        v = (uu * uu).astype(f)
        return (cond.astype(f) * v).astype(f)

    op = _register_op(name, Spec(body=body, reference=_ref))
    # Inject the perf-mode table: REGULAR = lower()'s 1x program; 2X_2P = the
    # hand-written two-chain program; 2X_1P/4X slots get the 1x program (they
    # require 16-bit dtypes, unreachable for fp32-src instructions).
    uops_1x = lower(op.spec, ver="v3")
    assert len(uops_1x) == 1
    spec_v3 = DveOpSpec(
        name=name,
        opcode=dve_ops.get_dve_sub_opcode(name),
        uops=uops_1x,
        uops_2x=[uops_1x[0]],
        uops_2x_2p=[_build_q2x_uop()],
        uops_4x=None,
        perf_max=2,
        rd1_en=False,
    )
    dve_ops._COMPILE_CACHE[(name, "v3")] = spec_v3
    return op


def build_nc(
    K=8,
    tfrac=0.72,
    pfrac=0.0,
    num_devices=N_CORES,
    bufs=4,
    reps=1,
    out_dt="bf16",
):
    """Build the per-core Bass program.

    K: batch rows per chunk.  tfrac: fraction of chunks routed through the
    ACT-assisted route B; pfrac: fraction through the Pool-assisted route P
    (both produce t' for the shared 2x DVE windowing op).  out_dt: device
    output dtype ("bf16" or "f32")."""
    assert B_SHARD % K == 0
    n_chunks = B_SHARD // K
    n_tot = 2 * n_chunks
    OUT_DT = BF16 if out_dt == "bf16" else F32

    nc = bacc.Bacc(
        "TRN2",
        target_bir_lowering=False,
        debug=False,
        enable_asserts=True,
        num_devices=num_devices,
    )
    xts_d = nc.dram_tensor("xts", [M, B_SHARD], F32, kind="ExternalInput")
    phs_d = nc.dram_tensor("phs", [M, L], F32, kind="ExternalInput")
    ivq_d = nc.dram_tensor("ivq", [M], F32, kind="ExternalInput")
    ivqs_d = nc.dram_tensor("ivqs", [M], F32, kind="ExternalInput")
    y_d = nc.dram_tensor("out", [B_SHARD, M * L], OUT_DT, kind="ExternalOutput")
    # out[k, (h*128+i)*256 + j] viewed as [h, i(part), k, j]
    yr = y_d.ap().rearrange("b (h i j) -> h i b j", h=2, i=HALF, j=L)
    ivr = ivq_d.ap().rearrange("(h i one) -> h i one", h=2, one=1)
    ivsr = ivqs_d.ap().rearrange("(h i one) -> h i one", h=2, one=1)
    xtr = xts_d.ap().rearrange("(h i) b -> h i b", h=2)
    phr = phs_d.ap().rearrange("(h i) j -> h i j", h=2)

    winpoly_xc = _get_winpoly_xc_op()
    winpoly_q = _get_winpoly_q_op() if (tfrac > 0 or pfrac > 0) else None

    # Proportional interleave of routes among the global chunk sequence.
    # Lead with route-A (pure DVE) chunks: engine queues are in-order, so a
    # B-chunk first would stall DVE behind ACT's first K Squares (the sim
    # trace showed an 8.4us DVE gap at kernel start).
    n_B = int(round(tfrac * n_tot))
    n_P = int(round(pfrac * n_tot))
    n_A = n_tot - n_B - n_P
    assert n_A >= 0
    slots = []
    for label, cnt, phase in (("A", n_A, 0.25), ("B", n_B, 0.75), ("P", n_P, 0.5)):
        slots += [((j + phase) / cnt, label) for j in range(cnt)]
    routes = [lab for _, lab in sorted(slots)]

    with TileContext(nc) as tc:
        with (
            tc.tile_pool(name="const", bufs=1) as cpool,
            tc.tile_pool(name="twork", bufs=4) as tpool,
            tc.tile_pool(name="owork", bufs=bufs) as opool,
        ):
            if tfrac > 0:
                # Trigger the Square table-set load while input DMAs fly.
                warm_t = cpool.tile([HALF, 1], F32, tag="warm")
                nc.gpsimd.memset(warm_t[:], 0.0)
                warm2 = cpool.tile([HALF, 1], F32, tag="warm2")
                nc.scalar.activation(
                    warm2[:], warm_t[:], mybir.ActivationFunctionType.Square,
                    bias=0.0, scale=1.0,
                )
            # Split input loads across the SP and ACT DMA rings so the
            # sequencers issue them concurrently (~565 ns serial issue each).
            ph_t, iv_t, ivs_t, xt_t = [], [], [], []
            for h in range(2):
                eng = nc.sync if h == 0 else nc.scalar
                p = cpool.tile([HALF, L], F32, tag=f"ph{h}")
                eng.dma_start(out=p[:], in_=phr[h])
                ph_t.append(p)
                i_ = cpool.tile([HALF, 1], F32, tag=f"iv{h}")
                eng.dma_start(out=i_[:], in_=ivr[h])
                iv_t.append(i_)
                xt = cpool.tile([HALF, B_SHARD], F32, tag=f"xt{h}")
                eng.dma_start(out=xt[:], in_=xtr[h])
                xt_t.append(xt)
                # ivqs is only needed by route-B chunks -- load it last so the
                # first xc chunk's deps (ph, iv, xt) land earlier.
                i2 = cpool.tile([HALF, 1], F32, tag=f"ivs{h}")
                eng.dma_start(out=i2[:], in_=ivsr[h])
                ivs_t.append(i2)

            mult, add = mybir.AluOpType.mult, mybir.AluOpType.add

            def emit_chunk(h, ci, route):
                o = opool.tile([HALF, K * L], OUT_DT, tag="o")
                if route in ("B", "P"):
                    t = tpool.tile([HALF, K * L], F32, tag="t")
                    if route == "B":
                        for k in range(K):
                            kg = ci * K + k
                            nc.scalar.activation(
                                t[:, k * L : (k + 1) * L],
                                ph_t[h][:],
                                mybir.ActivationFunctionType.Square,
                                bias=xt_t[h][:, kg : kg + 1],
                                scale=-1.0,
                            )
                    else:  # Pool: d per row, then one big square
                        d = tpool.tile([HALF, K * L], F32, tag="d")
                        for k in range(K):
                            kg = ci * K + k
                            nc.gpsimd.tensor_scalar(
                                out=d[:, k * L : (k + 1) * L],
                                in0=ph_t[h][:],
                                scalar1=-1.0,
                                scalar2=xt_t[h][:, kg : kg + 1],
                                op0=mult,
                                op1=add,
                            )
                        nc.gpsimd.tensor_tensor(
                            out=t[:], in0=d[:], in1=d[:], op=mult
                        )
                    bi = nc.vector._custom_dve(
                        winpoly_q,
                        out=o[:],
                        in0=t[:],
                        s0=ivs_t[h][:],
                        s1=0.0,
                        imm2=C_SQ32,
                    )
                    bi.ins.perf_max = 2
                else:
                    nc.vector._custom_dve(
                        winpoly_xc,
                        out=o[:],
                        in0=ph_t[h][:].unsqueeze(1).broadcast_to([HALF, K, L]),
                        in1=xt_t[h][:, ci * K : (ci + 1) * K],
                        s0=iv_t[h][:],
                        s1=0.0,
                        imm2=C_SQ32,
                    )
                nc.sync.dma_start(out=yr[h, :, ci * K : (ci + 1) * K, :], in_=o[:])

            import contextlib

            loop_ctx = (
                tc.For_i(0, reps, 1, hint_engines=tuple(mybir.ALL_ENGINES))
                if reps > 1
                else contextlib.nullcontext()
            )
            with loop_ctx:
                for h in range(2):
                    for ci in range(n_chunks):
                        emit_chunk(h, ci, routes[h * n_chunks + ci])
    nc.compile()
    return nc


_NC_CACHE = {}


def _build_cfg():
    K = int(os.environ.get("HB_K", "16"))
    tfrac = float(os.environ.get("HB_TFRAC", "0.5"))
    pfrac = float(os.environ.get("HB_PFRAC", "0.0"))
    out_dt = os.environ.get("HB_OUT_DT", "bf16")
    bufs = int(os.environ.get("HB_BUFS", "4"))
    return dict(K=K, tfrac=tfrac, pfrac=pfrac, out_dt=out_dt, bufs=bufs)


def _get_nc():
    key = tuple(sorted(_build_cfg().items()))
    if key not in _NC_CACHE:
        _NC_CACHE[key] = build_nc(**_build_cfg())
    return _NC_CACHE[key]


def make_in_maps(x, phis, interval):
    """Host-side pre-scaling + sharding. Returns per-core input maps."""
    x = np.asarray(x, dtype=np.float32)
    phis = np.asarray(phis, dtype=np.float32)
    interval = np.asarray(interval, dtype=np.float32)
    phs = np.ascontiguousarray(phis * np.float32(SCALE))
    ivq = np.ascontiguousarray(
        (interval.astype(np.float64) * SCALE) ** 2
    ).astype(np.float32)
    ivqs = (ivq - np.float32(C_SQ32)).astype(np.float32)
    in_maps = []
    for c in range(N_CORES):
        shard = x[c * B_SHARD : (c + 1) * B_SHARD]
        in_maps.append(
            {
                "xts": np.ascontiguousarray(shard.T * np.float32(SCALE)),
                "phs": phs,
                "ivq": ivq,
                "ivqs": ivqs,
            }
        )
    return in_maps


def kernel(x, phis, interval):
    x = np.ascontiguousarray(x, dtype=np.float32)
    phis = np.ascontiguousarray(phis, dtype=np.float32)
    interval = np.ascontiguousarray(interval, dtype=np.float32)
    assert x.shape == (B, M) and phis.shape == (M, L) and interval.shape == (M,)

    nc = _get_nc()
    in_maps = make_in_maps(x, phis, interval)
    res = run_bass_kernel_spmd(nc, in_maps, core_ids=list(range(N_CORES)))
    # Device emits v = val - 1/4 inside the window, 0 outside; restore +1/4.
    dev = np.concatenate(
        [np.asarray(res.results[c]["out"]) for c in range(N_CORES)], axis=0
    ).astype(np.float32)
    return np.where(dev != 0.0, dev + np.float32(C_QTR), np.float32(0.0))


# revision 32
# speedup vs baseline: 1.0617x; 1.0617x over previous
"""Trainium2 Bass kernel for histogram_binning (windowed-cosine binning).

Reference computation (per element):
    d = x[k,i] - phis[i,j]
    out[k, i*L+j] = 0.5*cos(d)+0.5  if  -interval[i] < d <= interval[i]  else 0

Strategy (8 cores, data-parallel over batch; each core owns 128 batch rows):
  - Polynomial value path (no ACT sin): with s = 48**-0.25 and t' = (s*d)^2,
        0.5*cos(d)+0.5  ~=  (t' - sqrt(3)/2)^2 + 1/4         (|err| <= 7e-4)
    and the window test becomes  t' <= (s*iv)^2  (exact up to ~5e-7 relative
    slack at the window edge; the harness tolerance is 2e-2 L2).
    Host pre-scales the tiny inputs: xts = s*x^T, phs = s*phis, ivq = (s*iv)^2.
  - On-chip layout: partition dim = feature i (two 128-halves), free dim =
    (k_block, j).  phs half [128,256], ivq half [128,1], xts half [128,128]
    stay resident.
  - Two fused compute routes per chunk (mixed to balance DVE vs ACT):
    route A (DVE-only, "xc"): ONE custom DVE instruction per K-row chunk
      with a hand-written 3-state subdim FSM: in0 streams the phi tile K
      times via a stride-0 page dim; block 0's swap flop holds the current
      row's x (reloaded from in1 at each SUB_DIM_DONE); computes
      out = (t'<=ivq) * (t'-sqrt3/2)^2 fused.  ~304 ns/row vs ~553 for
      per-row ops (amortizes the ~280 ns DVE instruction overhead).
    route B (ACT-assisted): ACT Square computes t' = (xs - phi_s)^2 per row
      (bias = xs column, scale = -1); one 4-stage custom DVE op with a
      hand-written 2x_2P perf-mode program (2 elem/cycle, write0=chainA /
      write1=chainB like the stock fp32 tensor_scalar 2x program) windows +
      polys the whole chunk: out = (u<=ivq-sqrt3/2) * u^2, u = t'-sqrt3/2.
  - Device output is v = val - 1/4 (in-window) or 0; the host restores the
    +1/4 under the (out != 0) mask.  Output dtype is bf16 (halves the
    HBM-write floor; +2e-3 rel err), upcast to f32 on the host.
  - Measured: ~56 us vs 115.3 us baseline (rel err ~1.3e-3, gate 2e-2).
"""

import math
import os

import numpy as np

import concourse.bacc as bacc
import concourse.mybir as mybir
from concourse import dve_ops
from concourse.bass_utils import run_bass_kernel_spmd
from concourse.dve_spec import (
    C0,
    C1,
    C2,
    C3,
    Spec,
    Src0,
    _has_src1,
    _spill_c3_to_src1,
    lower,
    sq,
)
from concourse.dve_uop import (
    DISABLE,
    ENABLE,
    AluInp,
    AluOp,
    DelayInp,
    DveOpSpec,
    InpSel,
    OutPath,
    OutSel,
    Trigger,
    UopConfig,
    UopDpConfig,
)
from concourse.tile import TileContext

B, M, L = 1024, 256, 256
N_CORES = 8
B_SHARD = B // N_CORES  # 128
HALF = 128  # features per partition-half
F32 = mybir.dt.float32
BF16 = mybir.dt.bfloat16

SCALE = float(48.0 ** -0.25)  # s: makes the poly monic in t'
C_SQ32 = float(math.sqrt(3.0) / 2.0)  # imm2
C_QTR = 0.25  # s1

_OPS_CACHE = {}


def _register_op(name, spec, subdim=False):
    """Register a custom DVE op under `name`, computing its uops sha."""
    if name in _OPS_CACHE:
        return _OPS_CACHE[name]
    for existing in dve_ops.OPS:
        if existing.name == name:
            _OPS_CACHE[name] = existing
            return existing
    if name not in dve_ops._SUB_OPCODE_FOR_NAME:
        row = max(dve_ops._SUB_OPCODE_FOR_NAME.values()) + 1
        assert row < 0x20, "no free custom-DVE opcode rows"
        dve_ops._SUB_OPCODE_FOR_NAME[name] = row
    shas = {}
    for ver in ("v3", "v4"):
        uops = lower(spec, ver=ver)
        shas[ver] = DveOpSpec(
            name=name,
            opcode=dve_ops.get_dve_sub_opcode(name),
            uops=uops,
            rd1_en=_has_src1(spec),
        ).sha(ver)
    op = dve_ops.DveOp(name, spec, subdim=subdim, uops_sha=shas)
    dve_ops.OPS.append(op)
    dve_ops.CUSTOM_DVE_SPECS[name] = spec
    _OPS_CACHE[name] = op
    return op


def _get_winpoly_full_op():
    """Fully fused per-row op.  Src0 = s*phi, C3 (in1, [P,1]) = s*x,
    C0 = (s*iv)^2 [P,1], C2 (imm2) = sqrt(3)/2.
        d  = C3 - Src0            t' = d^2
        out = (t' <= C0) * (t' - C2)^2        (the +1/4 is host-side)
    6 ALU stages."""
    d = C3 - Src0
    t = sq(d)
    cond = t <= C0
    body = cond * sq(t - C2)
    body = _spill_c3_to_src1(body)

    def _ref(in0, in1, s0, s1, imm2):
        f = np.float32
        d = (in1 - in0).astype(f)
        t = (d * d).astype(f)
        cond = t <= s0
        u = (t - f(imm2)).astype(f)
        v = (u * u).astype(f)
        return (cond.astype(f) * v).astype(f)

    return _register_op("WINPOLY_VMQ_ANT", Spec(body=body, reference=_ref))


def _get_winpoly_vmq2_op():
    """Latch-free fused per-row op (single uop state, rd1 free).
    Src0 = s*phi, C1 (s1, [P,1]) = s*x, C0 = (s*iv)^2 [P,1],
    C2 (imm2) = sqrt(3)/2.
        d = C1 - Src0 ; t' = d^2 ; out = (t' <= C0) * (t' - C2)^2
    6 ALU stages; the +1/4 is restored host-side."""
    d = C1 - Src0
    t = sq(d)
    cond = t <= C0
    body = cond * sq(t - C2)

    def _ref(in0, in1, s0, s1, imm2):
        f = np.float32
        d = (s1 - in0).astype(f)
        t = (d * d).astype(f)
        cond = t <= s0
        u = (t - f(imm2)).astype(f)
        v = (u * u).astype(f)
        return (cond.astype(f) * v).astype(f)

    return _register_op("WINPOLY_VMQ2_ANT", Spec(body=body, reference=_ref))


def _build_xc_uops():
    """3-state FSM for the x-cycling fused op (subdim pages over in0).

    in0 = phi tile streamed K times via a stride-0 page dim [P, K, N];
    in1 = [P, K] per-page x values.  Block 0's swap flop holds the current
    page's x; SUB_DIM_DONE hops to a step state that reloads it from Src1
    while processing that page's first element.
        d = x - phi ; t = d^2 ; out = (t <= C0) * (t - C2)^2
    """
    P, SW = AluInp.PREV_ALU_OUT, AluInp.CURR_SWAP_OUT
    D = [AluInp.PREV_DELAY_0, AluInp.PREV_DELAY_1, AluInp.PREV_DELAY_2,
         AluInp.PREV_DELAY_3, AluInp.PREV_DELAY_4, AluInp.PREV_DELAY_5]

    # state 0: init -- swap-load x[0] from Src1, no compute, no output
    init = UopConfig()
    init.enable_input(InpSel.SRC_1, 1)  # -> lane 0
    init.require_inp1 = ENABLE
    init.trigger = (Trigger.COUNT, Trigger.NONE, Trigger.NONE)
    init.repeat_count = 1
    init.next_uop = (1, 0, 0)
    b = init.datapath_config
    b[0].enable_alu(AluOp.BYPASS, D[0], D[0])
    b[0].swap_enable = ENABLE

    # state 1: steady -- d = swap - Src0, windowed poly, write out
    st = UopConfig()
    st.enable_input(InpSel.SRC_0, 0)
    st.enable_input(InpSel.CONST_0, 1)  # ivq -> lane 0
    st.enable_input(InpSel.CONST_2, 2)  # sqrt3/2 -> lane 1
    st.require_inp0 = ENABLE
    st.trigger = (Trigger.SRC_TENSOR_DONE, Trigger.SUB_DIM_DONE, Trigger.NONE)
    st.next_uop = (0, 2, 0)
    st.enable_output(OutSel.ALU_OUT, OutPath.WR0_LO)
    b = st.datapath_config
    b[0].enable_alu(AluOp.SUBTRACT, SW, P).pass_through_delay(0, 1)
    b[1].enable_alu(AluOp.MULTIPLY, P, P).pass_through_delay(0, 1)
    b[2].enable_alu(AluOp.IS_GE, D[0], P).pass_through_delay(1)
    b[2].enable_delay_from_src(DelayInp.PREV_ALU_OUT, 2)  # t
    b[3].enable_alu(AluOp.SUBTRACT, D[2], D[1])
    b[3].enable_delay_from_src(DelayInp.PREV_ALU_OUT, 3)  # cond
    b[4].enable_alu(AluOp.MULTIPLY, P, P).pass_through_delay(3)
    b[5].enable_alu(AluOp.MULTIPLY, P, D[3])
    b[6].pass_through_alu()
    b[7].pass_through_alu()

    # state 2: step -- swap-load next x AND process the page's first element
    sp = UopConfig()
    sp.enable_input(InpSel.SRC_1, 1)   # x  -> lane 0
    sp.enable_input(InpSel.SRC_0, 2)   # phi -> lane 1
    sp.enable_input(InpSel.CONST_0, 3)  # ivq -> lane 2
    sp.enable_input(InpSel.CONST_2, 4)  # sqrt3/2 -> lane 3
    sp.require_inp0 = ENABLE
    sp.require_inp1 = ENABLE
    sp.trigger = (Trigger.SRC_TENSOR_DONE, Trigger.SUB_DIM_DONE, Trigger.COUNT)
    sp.next_uop = (0, 2, 1)
    sp.repeat_count = 1
    sp.enable_output(OutSel.ALU_OUT, OutPath.WR0_LO)
    b = sp.datapath_config
    b[0].enable_alu(AluOp.BYPASS, D[0], D[0]).pass_through_delay(1, 2, 3)
    b[0].swap_enable = ENABLE
    b[1].enable_alu(AluOp.SUBTRACT, P, D[1]).pass_through_delay(2, 3)
    b[2].enable_alu(AluOp.MULTIPLY, P, P).pass_through_delay(2, 3)
    b[3].enable_alu(AluOp.IS_GE, D[2], P).pass_through_delay(3)
    b[3].enable_delay_from_src(DelayInp.PREV_ALU_OUT, 4)  # t
    b[4].enable_alu(AluOp.SUBTRACT, D[4], D[3])
    b[4].enable_delay_from_src(DelayInp.PREV_ALU_OUT, 5)  # cond
    b[5].enable_alu(AluOp.MULTIPLY, P, P).pass_through_delay(5)
    b[6].enable_alu(AluOp.MULTIPLY, P, D[5])
    b[7].pass_through_alu()
    return [init, st, sp]


def _get_winpoly_xc_op():
    """x-cycling fused op: one instruction per K-row chunk.
    in0 = s*phi [P, K(stride-0), N], in1 = s*x [P, K] (one per page),
    C0 = (s*iv)^2 [P,1], C2 (imm2) = sqrt(3)/2.
        d = x_page - phi ; t' = d^2 ; out = (t' <= C0) * (t' - C2)^2"""
    name = "WINPOLY_XC_ANT"
    # Lowerable stand-in body with the same leaf set (the runtime table is
    # the hand-written FSM injected below).
    d = C3 - Src0
    t = sq(d)
    body = _spill_c3_to_src1((t <= C0) * sq(t - C2))

    def _ref(in0, in1, s0, s1, imm2):
        f = np.float32
        a0 = np.asarray(in0, dtype=f)
        x = np.asarray(in1, dtype=f)
        if a0.ndim == 3:
            Pd, S, N = a0.shape
            xx = x.reshape(Pd, S, 1)
            c0 = np.asarray(s0, dtype=f).reshape(Pd, 1, 1) if np.ndim(s0) else f(s0)
        else:
            xx = x
            c0 = s0
        dd = (xx - a0).astype(f)
        tt = (dd * dd).astype(f)
        cond = tt <= c0
        u = (tt - f(imm2)).astype(f)
        v = (u * u).astype(f)
        return (cond.astype(f) * v).astype(f)

    op = _register_op(name, Spec(body=body, reference=_ref), subdim=True)
    spec_v3 = DveOpSpec(
        name=name,
        opcode=dve_ops.get_dve_sub_opcode(name),
        uops=_build_xc_uops(),
        rd1_en=True,
    )
    dve_ops._COMPILE_CACHE[(name, "v3")] = spec_v3
    return op


def _get_winpoly_t_op():
    """Window + poly from precomputed t' (Src0).  C0 = (s*iv)^2 [P,1],
    C2 (imm2) = sqrt(3)/2, C1 (s1) = 1/4.  5 ALU stages."""
    cond = Src0 <= C0
    body = cond * (sq(Src0 - C2) + C1)

    def _ref(in0, in1, s0, s1, imm2):
        f = np.float32
        cond = in0 <= s0
        u = (in0 - f(imm2)).astype(f)
        w = ((u * u).astype(f) + f(s1)).astype(f)
        return (cond.astype(f) * w).astype(f)

    return _register_op("WINPOLY_T_ANT", Spec(body=body, reference=_ref))


def _build_q2x_uop():
    """Hand-written 2x_2P (two results/cycle) program for WINPOLY_Q_ANT.

    Per cycle the engine reads in0[2i] via rd0 (-> SRC_0) and in0[2i+1] via
    rd1 (-> SRC_1), same convention as the stock fp32 tensor_scalar 2x_2P
    program (write0_lo = chain-A result, write1_lo = chain-B result).

    Chain A (blocks 0-3) and chain B (blocks 4-7) each compute
        u = elem - C2 ; cond = (u <= C0) ; v = u*u ; out = cond * v
    Lane plan:
      lane0: elemB (SRC_1 via inp1), consumed at b4
      lane1: C0 (inp2), read at b1 and b5
      lane2: C2 (inp3), read at b0 and b4
      lane3: uA (cap b1, read b2); condB (cap b6, read b7)
      lane4: condA (cap b2, read b3); outA (cap b4, ride to write mux)
      lane5: uB (cap b5, read b6)
    """
    u = UopConfig()
    u.enable_input(InpSel.SRC_0, 0)
    u.enable_input(InpSel.SRC_1, 1)   # -> lane 0
    u.enable_input(InpSel.CONST_0, 2)  # -> lane 1
    u.enable_input(InpSel.CONST_2, 3)  # -> lane 2
    u.require_inp0 = ENABLE
    u.require_inp1 = ENABLE
    u.trigger = (Trigger.SRC_TENSOR_DONE, Trigger.NONE, Trigger.NONE)
    u.next_uop = (0, 0, 0)
    b = u.datapath_config
    P = AluInp.PREV_ALU_OUT
    D = [AluInp.PREV_DELAY_0, AluInp.PREV_DELAY_1, AluInp.PREV_DELAY_2,
         AluInp.PREV_DELAY_3, AluInp.PREV_DELAY_4, AluInp.PREV_DELAY_5]
    # b0: uA = SRC_0 - C2
    b[0].enable_alu(AluOp.SUBTRACT, P, D[2]).pass_through_delay(0, 1, 2)
    # b1: condA = (C0 >= uA); capture uA -> lane3
    b[1].enable_alu(AluOp.IS_GE, D[1], P).pass_through_delay(0, 1, 2)
    b[1].enable_delay_from_src(DelayInp.PREV_ALU_OUT, 3)
    # b2: vA = uA * uA; capture condA -> lane4
    b[2].enable_alu(AluOp.MULTIPLY, D[3], D[3]).pass_through_delay(0, 1, 2)
    b[2].enable_delay_from_src(DelayInp.PREV_ALU_OUT, 4)
    # b3: outA = vA * condA
    b[3].enable_alu(AluOp.MULTIPLY, P, D[4]).pass_through_delay(0, 1, 2)
    # b4: uB = elemB - C2; capture outA -> lane4
    b[4].enable_alu(AluOp.SUBTRACT, D[0], D[2]).pass_through_delay(1)
    b[4].enable_delay_from_src(DelayInp.PREV_ALU_OUT, 4)
    # b5: condB = (C0 >= uB); capture uB -> lane5
    b[5].enable_alu(AluOp.IS_GE, D[1], P).pass_through_delay(4)
    b[5].enable_delay_from_src(DelayInp.PREV_ALU_OUT, 5)
    # b6: vB = uB * uB; capture condB -> lane3
    b[6].enable_alu(AluOp.MULTIPLY, D[5], D[5]).pass_through_delay(4)
    b[6].enable_delay_from_src(DelayInp.PREV_ALU_OUT, 3)
    # b7: outB = vB * condB; outA rides lane4 to the write mux
    b[7].enable_alu(AluOp.MULTIPLY, P, D[3]).pass_through_delay(4)
    u.enable_output(OutSel.DELAY_4, OutPath.WR0_LO)  # result A
    u.enable_output(OutSel.ALU_OUT, OutPath.WR1_LO)  # result B
    return u


def _get_winpoly_q_op():
    """Window + poly-minus-quarter from t' (Src0), 4 ALU stages, with a
    hand-written 2x_2P perf-mode program (2 elem/cycle for fp32 SBUF src).
        u = Src0 - C2 ; out = (u <= C0) * u^2
    C0 = (s*iv)^2 - sqrt(3)/2 [P,1] (shifted window bound; compare on u is
    equivalent to t' <= (s*iv)^2 by monotonicity), C2 (imm2) = sqrt(3)/2.
    The missing +1/4 is restored on the host (out != 0 marks the window)."""
    name = "WINPOLY_Q_ANT"
    u = Src0 - C2
    cond = u <= C0
    body = cond * sq(u)

    def _ref(in0, in1, s0, s1, imm2):
        f = np.float32
        uu = (in0 - f(imm2)).astype(f)
        cond = uu <= s0
        v = (uu * uu).astype(f)
        return (cond.astype(f) * v).astype(f)

    op = _register_op(name, Spec(body=body, reference=_ref))
    # Inject the perf-mode table: REGULAR = lower()'s 1x program; 2X_2P = the
    # hand-written two-chain program; 2X_1P/4X slots get the 1x program (they
    # require 16-bit dtypes, unreachable for fp32-src instructions).
    uops_1x = lower(op.spec, ver="v3")
    assert len(uops_1x) == 1
    spec_v3 = DveOpSpec(
        name=name,
        opcode=dve_ops.get_dve_sub_opcode(name),
        uops=uops_1x,
        uops_2x=[uops_1x[0]],
        uops_2x_2p=[_build_q2x_uop()],
        uops_4x=None,
        perf_max=2,
        rd1_en=False,
    )
    dve_ops._COMPILE_CACHE[(name, "v3")] = spec_v3
    return op


def build_nc(
    K=8,
    tfrac=0.72,
    pfrac=0.0,
    num_devices=N_CORES,
    bufs=4,
    reps=1,
    out_dt="bf16",
):
    """Build the per-core Bass program.

    K: batch rows per chunk.  tfrac: fraction of chunks routed through the
    ACT-assisted route B; pfrac: fraction through the Pool-assisted route P
    (both produce t' for the shared 2x DVE windowing op).  out_dt: device
    output dtype ("bf16" or "f32")."""
    assert B_SHARD % K == 0
    n_chunks = B_SHARD // K
    n_tot = 2 * n_chunks
    OUT_DT = BF16 if out_dt == "bf16" else F32

    nc = bacc.Bacc(
        "TRN2",
        target_bir_lowering=False,
        debug=False,
        enable_asserts=True,
        num_devices=num_devices,
    )
    xts_d = nc.dram_tensor("xts", [M, B_SHARD], F32, kind="ExternalInput")
    phs_d = nc.dram_tensor("phs", [M, L], F32, kind="ExternalInput")
    ivq_d = nc.dram_tensor("ivq", [M], F32, kind="ExternalInput")
    ivqs_d = nc.dram_tensor("ivqs", [M], F32, kind="ExternalInput")
    y_d = nc.dram_tensor("out", [B_SHARD, M * L], OUT_DT, kind="ExternalOutput")
    # out[k, (h*128+i)*256 + j] viewed as [h, i(part), k, j]
    yr = y_d.ap().rearrange("b (h i j) -> h i b j", h=2, i=HALF, j=L)
    ivr = ivq_d.ap().rearrange("(h i one) -> h i one", h=2, one=1)
    ivsr = ivqs_d.ap().rearrange("(h i one) -> h i one", h=2, one=1)
    xtr = xts_d.ap().rearrange("(h i) b -> h i b", h=2)
    phr = phs_d.ap().rearrange("(h i) j -> h i j", h=2)

    winpoly_xc = _get_winpoly_xc_op()
    winpoly_q = _get_winpoly_q_op() if (tfrac > 0 or pfrac > 0) else None

    # Proportional interleave of routes among the global chunk sequence.
    # Lead with route-A (pure DVE) chunks: engine queues are in-order, so a
    # B-chunk first would stall DVE behind ACT's first K Squares (the sim
    # trace showed an 8.4us DVE gap at kernel start).
    n_B = int(round(tfrac * n_tot))
    n_P = int(round(pfrac * n_tot))
    n_A = n_tot - n_B - n_P
    assert n_A >= 0
    slots = []
    for label, cnt, phase in (("A", n_A, 0.25), ("B", n_B, 0.75), ("P", n_P, 0.5)):
        slots += [((j + phase) / cnt, label) for j in range(cnt)]
    routes = [lab for _, lab in sorted(slots)]

    with TileContext(nc) as tc:
        with (
            tc.tile_pool(name="const", bufs=1) as cpool,
            tc.tile_pool(name="twork", bufs=(3 if K >= 32 else 4)) as tpool,
            tc.tile_pool(name="owork", bufs=bufs) as opool,
        ):
            if tfrac > 0:
                # Trigger the Square table-set load while input DMAs fly.
                warm_t = cpool.tile([HALF, 1], F32, tag="warm")
                nc.gpsimd.memset(warm_t[:], 0.0)
                warm2 = cpool.tile([HALF, 1], F32, tag="warm2")
                nc.scalar.activation(
                    warm2[:], warm_t[:], mybir.ActivationFunctionType.Square,
                    bias=0.0, scale=1.0,
                )
            # Split input loads across the SP and ACT DMA rings so the
            # sequencers issue them concurrently (~565 ns serial issue each).
            ph_t, iv_t, ivs_t, xt_t = [], [], [], []
            for h in range(2):
                eng = nc.sync if h == 0 else nc.scalar
                p = cpool.tile([HALF, L], F32, tag=f"ph{h}")
                eng.dma_start(out=p[:], in_=phr[h])
                ph_t.append(p)
                i_ = cpool.tile([HALF, 1], F32, tag=f"iv{h}")
                eng.dma_start(out=i_[:], in_=ivr[h])
                iv_t.append(i_)
                xt = cpool.tile([HALF, B_SHARD], F32, tag=f"xt{h}")
                eng.dma_start(out=xt[:], in_=xtr[h])
                xt_t.append(xt)
                # ivqs is only needed by route-B chunks -- load it last so the
                # first xc chunk's deps (ph, iv, xt) land earlier.
                i2 = cpool.tile([HALF, 1], F32, tag=f"ivs{h}")
                eng.dma_start(out=i2[:], in_=ivsr[h])
                ivs_t.append(i2)

            mult, add = mybir.AluOpType.mult, mybir.AluOpType.add

            def emit_chunk(h, ci, route):
                o = opool.tile([HALF, K * L], OUT_DT, tag="o")
                if route in ("B", "P"):
                    t = tpool.tile([HALF, K * L], F32, tag="t")
                    if route == "B":
                        for k in range(K):
                            kg = ci * K + k
                            nc.scalar.activation(
                                t[:, k * L : (k + 1) * L],
                                ph_t[h][:],
                                mybir.ActivationFunctionType.Square,
                                bias=xt_t[h][:, kg : kg + 1],
                                scale=-1.0,
                            )
                    else:  # Pool: d per row, then one big square
                        d = tpool.tile([HALF, K * L], F32, tag="d")
                        for k in range(K):
                            kg = ci * K + k
                            nc.gpsimd.tensor_scalar(
                                out=d[:, k * L : (k + 1) * L],
                                in0=ph_t[h][:],
                                scalar1=-1.0,
                                scalar2=xt_t[h][:, kg : kg + 1],
                                op0=mult,
                                op1=add,
                            )
                        nc.gpsimd.tensor_tensor(
                            out=t[:], in0=d[:], in1=d[:], op=mult
                        )
                    bi = nc.vector._custom_dve(
                        winpoly_q,
                        out=o[:],
                        in0=t[:],
                        s0=ivs_t[h][:],
                        s1=0.0,
                        imm2=C_SQ32,
                    )
                    bi.ins.perf_max = 2
                else:
                    nc.vector._custom_dve(
                        winpoly_xc,
                        out=o[:],
                        in0=ph_t[h][:].unsqueeze(1).broadcast_to([HALF, K, L]),
                        in1=xt_t[h][:, ci * K : (ci + 1) * K],
                        s0=iv_t[h][:],
                        s1=0.0,
                        imm2=C_SQ32,
                    )
                nc.sync.dma_start(out=yr[h, :, ci * K : (ci + 1) * K, :], in_=o[:])

            import contextlib

            loop_ctx = (
                tc.For_i(0, reps, 1, hint_engines=tuple(mybir.ALL_ENGINES))
                if reps > 1
                else contextlib.nullcontext()
            )
            with loop_ctx:
                for h in range(2):
                    for ci in range(n_chunks):
                        emit_chunk(h, ci, routes[h * n_chunks + ci])
    nc.compile()
    return nc


_NC_CACHE = {}


def _build_cfg():
    K = int(os.environ.get("HB_K", "16"))
    tfrac = float(os.environ.get("HB_TFRAC", "0.5"))
    pfrac = float(os.environ.get("HB_PFRAC", "0.0"))
    out_dt = os.environ.get("HB_OUT_DT", "bf16")
    bufs = int(os.environ.get("HB_BUFS", "4"))
    return dict(K=K, tfrac=tfrac, pfrac=pfrac, out_dt=out_dt, bufs=bufs)


def _get_nc():
    key = tuple(sorted(_build_cfg().items()))
    if key not in _NC_CACHE:
        _NC_CACHE[key] = build_nc(**_build_cfg())
    return _NC_CACHE[key]


def make_in_maps(x, phis, interval):
    """Host-side pre-scaling + sharding. Returns per-core input maps."""
    x = np.asarray(x, dtype=np.float32)
    phis = np.asarray(phis, dtype=np.float32)
    interval = np.asarray(interval, dtype=np.float32)
    phs = np.ascontiguousarray(phis * np.float32(SCALE))
    ivq = np.ascontiguousarray(
        (interval.astype(np.float64) * SCALE) ** 2
    ).astype(np.float32)
    ivqs = (ivq - np.float32(C_SQ32)).astype(np.float32)
    in_maps = []
    for c in range(N_CORES):
        shard = x[c * B_SHARD : (c + 1) * B_SHARD]
        in_maps.append(
            {
                "xts": np.ascontiguousarray(shard.T * np.float32(SCALE)),
                "phs": phs,
                "ivq": ivq,
                "ivqs": ivqs,
            }
        )
    return in_maps


def kernel(x, phis, interval):
    x = np.ascontiguousarray(x, dtype=np.float32)
    phis = np.ascontiguousarray(phis, dtype=np.float32)
    interval = np.ascontiguousarray(interval, dtype=np.float32)
    assert x.shape == (B, M) and phis.shape == (M, L) and interval.shape == (M,)

    nc = _get_nc()
    in_maps = make_in_maps(x, phis, interval)
    res = run_bass_kernel_spmd(nc, in_maps, core_ids=list(range(N_CORES)))
    # Device emits v = val - 1/4 inside the window, 0 outside; restore +1/4.
    dev = np.concatenate(
        [np.asarray(res.results[c]["out"]) for c in range(N_CORES)], axis=0
    ).astype(np.float32)
    return np.where(dev != 0.0, dev + np.float32(C_QTR), np.float32(0.0))
